# revision 1
# baseline (speedup 1.0000x reference)
"""Trainium2 Bass kernel for nn_CSpace: bank of 64 complex one-pole resonators
applied to audio forward and backward (FFT convolution in the reference).

Key idea: the resonator kernels are exact geometric sequences
k_c[t] = g_c * eig_c^t (|eig_c^24000| <= 1e-11 truncation error), so the
convolution is a first-order complex IIR filter. We compute it with
block-Toeplitz matmuls on the PE array:

  - time is split into 375 blocks of L=128; per (channel, direction, block
    group) one 128x125 @ (128x256) float32r matmul computes the within-block
    triangular convolution for both Re and Im planes at once,
  - the cross-block carry (IIR state at block boundaries) is computed via a
    trig-decomposed REAL first-order scan (tensor_tensor_scan) over per-block
    aggregates obtained from two small matmuls,
  - the carry contribution is added with a rank-2 matmul accumulating into
    the same PSUM tile,
  - backward direction reuses the same machinery on host-reversed views with
    column-flipped static matrices so outputs stream out in natural order.

Per core = one batch element: 64ch x 2dir x {Re,Im} x 48000 outputs.
"""
import sys

sys.path.insert(0, "/opt/trn_rl_repo")

import numpy as np

import concourse.bass as bass
import concourse.tile as tile
from concourse import mybir
from concourse.bass_utils import run_bass_kernel_spmd
from concourse.vector_clock import ScopedClock

# ---------------------------------------------------------------- constants
T = 48000
L = 128
NBLK = 375  # T / L
JG = 125  # blocks per group
NG = 3  # groups
C = 64  # channels
N_CORES = 8

RES_SIZE = 64
SR = 24000
MIN_FREQ = 10.0
MAX_FREQ = 12000.0
DECAY_FACTOR = 0.999
HIGH_FREQ_DECAY = 0.6

F32 = mybir.dt.float32
F32R = mybir.dt.float32r


class TC(tile.TileContext):
    """TileContext adapted to this walrus build: TPB instruction structs
    accept only ONE sync-wait command, so any scheduled instruction carrying
    more gets its extra waits split onto preceding same-engine nops (and the
    kernel-tail drain gets the same treatment)."""

    def _engine_for(self, engine):
        nc = self.nc
        m = {
            mybir.EngineType.PE: nc.tensor,
            mybir.EngineType.DVE: nc.vector,
            mybir.EngineType.Activation: nc.scalar,
            mybir.EngineType.Pool: nc.gpsimd,
            mybir.EngineType.SP: nc.sync,
        }
        return m[engine]

    def _commit_instruction(self, inst, lazy_reg_writes: bool = True):
        si = getattr(inst, "sync_info", None)
        if (
            si is not None
            and si.on_wait
            and len(si.on_wait) > 1
            and inst.engine in (
                mybir.EngineType.PE,
                mybir.EngineType.DVE,
                mybir.EngineType.Activation,
                mybir.EngineType.Pool,
                mybir.EngineType.SP,
            )
        ):
            waits = list(si.on_wait)
            cb = self.nc._state.pop_inst_callback()
            try:
                eng = self._engine_for(inst.engine)
                for w in waits[1:]:
                    nop = eng.nop()
                    nop.ins.sync_info = mybir.SyncInfo(
                        on_wait=[w], on_update=[]
                    )
            finally:
                self.nc._state.push_inst_callback(cb)
            si.on_wait = waits[:1]
        super()._commit_instruction(inst, lazy_reg_writes)

    def _drain_and_barrier(self, tick_clock, wait_clock):
        nc = self.nc
        probe = nc.sync.nop()
        wait_clock.add_sem_waits(
            probe.ins, ScopedClock({None: tick_clock.global_clock})
        )
        waits = list(probe.ins.sync_info.on_wait or [])
        probe.ins.sync_info.on_wait = waits[:1]
        for i in range(1, len(waits)):
            nop = nc.sync.nop()
            wait_clock.add_sem_waits(
                nop.ins, ScopedClock({None: tick_clock.global_clock})
            )
            nop.ins.sync_info.on_wait = waits[i : i + 1]
        nc.sync.drain()
        nc.all_engine_barrier()
        popped = nc._tile_sem_poison_stack.pop()
        assert popped is self._sem_poison
        nc.clear_and_free_semaphores(list(self.sems.allocated().values()))
        nc.all_engine_barrier()


# ---------------------------------------------------------------- host math
def _channel_params():
    freqs = np.logspace(np.log10(MIN_FREQ), np.log10(MAX_FREQ), RES_SIZE)
    decays = np.linspace(DECAY_FACTOR, HIGH_FREQ_DECAY, RES_SIZE)
    thetas = 2 * np.pi * freqs / SR
    eig = decays * np.exp(1j * thetas)
    gain = (1 - decays) / (1 - decays**SR)  # 1 / sum_t |eig|^t
    return eig, decays, thetas, gain


def build_static():
    """All data-independent arrays, float64 -> float32."""
    eig, r, thetas, gain = _channel_params()

    # powers[c, t] = eig_c^t for t in 0..L
    tpow = np.arange(L + 1)
    powers = eig[:, None] ** tpow[None, :]  # (C, L+1)

    i_idx = np.arange(L)
    dif = i_idx[:, None] - i_idx[None, :]  # (i, m)
    mask = dif >= 0
    gath = np.clip(dif, 0, None)

    TMF = np.zeros((C, L, 256), np.float64)
    TMB = np.zeros((C, L, 256), np.float64)
    EMF = np.zeros((2 * C, 256), np.float64)
    EMB = np.zeros((2 * C, 256), np.float64)
    for c in range(C):
        pw = powers[c][gath]  # (i, m) complex
        Tre = np.where(mask, gain[c] * pw.real, 0.0)
        Tim = np.where(mask, gain[c] * pw.imag, 0.0)
        TMF[c, :, :128] = Tre.T  # [m, i]
        TMF[c, :, 128:] = Tim.T
        TMB[c, :, :128] = Tre.T[:, ::-1]
        TMB[c, :, 128:] = Tim.T[:, ::-1]
        e = powers[c][1 : L + 1]  # eig^{i+1}, i = 0..127
        EMF[2 * c, :128], EMF[2 * c, 128:] = e.real, e.imag
        EMF[2 * c + 1, :128], EMF[2 * c + 1, 128:] = -e.imag, e.real
        eb = powers[c][L - np.arange(L)]  # eig^{128-i'}
        EMB[2 * c, :128], EMB[2 * c, 128:] = eb.real, eb.imag
        EMB[2 * c + 1, :128], EMB[2 * c + 1, 128:] = -eb.imag, eb.real

    m_idx = np.arange(L)
    AGG1 = np.zeros((L, 128), np.float64)
    AGG2 = np.zeros((L, 128), np.float64)
    for c in range(C):
        base = gain[c] * r[c] ** (127 - m_idx)
        cosm = np.cos(thetas[c] * m_idx)
        sinm = np.sin(thetas[c] * m_idx)
        AGG1[:, c] = base * cosm
        AGG1[:, 64 + c] = base * sinm
        AGG2[:, c] = base * sinm
        AGG2[:, 64 + c] = base * cosm

    j_idx = np.arange(NBLK)
    ang_b = thetas[:, None] * (128.0 * j_idx[None, :])  # (C, NBLK)
    ang_e = thetas[:, None] * (128.0 * j_idx[None, :] + 127.0)
    COSB, SINB = np.cos(ang_b), np.sin(ang_b)
    COSB2, SINB2 = np.cos(ang_e), np.sin(ang_e)
    A1 = np.concatenate([COSB, COSB], 0)
    A2 = np.concatenate([-SINB, SINB], 0)
    A1p = np.concatenate([SINB, -SINB], 0)
    A2p = np.concatenate([COSB, COSB], 0)
    B1 = np.concatenate([COSB2, -COSB2], 0)
    B2 = np.concatenate([SINB2, SINB2], 0)
    R128 = np.repeat((r**128)[None, :], 2, 0).reshape(2 * C, 1) * np.ones((1, NBLK))

    # Interleave chain rows so carries come out as (cr_c, ci_c) at rows
    # (2c, 2c+1): row 2c <- old row c, row 2c+1 <- old row 64+c. The same
    # permutation applies to AGG columns (they define the chain order).
    perm = np.empty(2 * C, np.int64)
    perm[0::2] = np.arange(C)
    perm[1::2] = np.arange(C) + C
    AGG1 = AGG1[:, perm]
    AGG2 = AGG2[:, perm]
    A1, A2, A1p, A2p = A1[perm], A2[perm], A1p[perm], A2p[perm]
    B1, B2, R128 = B1[perm], B2[perm], R128[perm]

    f32 = np.float32
    return {
        "TMF": np.ascontiguousarray(
            TMF.transpose(1, 0, 2).reshape(L, C * 256)
        ).astype(f32),
        "TMB": np.ascontiguousarray(
            TMB.transpose(1, 0, 2).reshape(L, C * 256)
        ).astype(f32),
        "EMF": EMF.astype(f32),
        "EMB": EMB.astype(f32),
        "AGG1": AGG1.astype(f32),
        "AGG2": AGG2.astype(f32),
        "A1": A1.astype(f32),
        "A2": A2.astype(f32),
        "A1p": A1p.astype(f32),
        "A2p": A2p.astype(f32),
        "B1": B1.astype(f32),
        "B2": B2.astype(f32),
        "R128": R128.astype(f32),
    }


def per_core_inputs(x):
    """x: (48000,) audio for one batch element."""
    XF = np.ascontiguousarray(x.reshape(NBLK, L).T).astype(np.float32)
    return {
        "XF": XF,
        "XBM": np.ascontiguousarray(XF[::-1, :]),
        "XBA": np.ascontiguousarray(XF[::-1, ::-1]),
    }


# ---------------------------------------------------------------- program
def build_nc(skip_rank2=False, skip_mains=False, skip_carries=False,
             nch=C, copy_engine="any", tm_bufs=12):
    nc = bass.Bass(
        trn_type="TRN2", target_bir_lowering=False, debug=False, num_devices=1
    )

    def din(name, shape, dt=F32):
        return nc.dram_tensor(name, list(shape), dt, kind="ExternalInput").ap()

    MM_INPUTS = {"XF", "XBM", "XBA", "AGG1", "AGG2", "TMF", "TMB", "EMF", "EMB"}
    d = {}
    for name, shape in [
        ("XF", (L, NBLK)),
        ("XBM", (L, NBLK)),
        ("XBA", (L, NBLK)),
        ("AGG1", (L, 128)),
        ("AGG2", (L, 128)),
        ("A1", (2 * C, NBLK)),
        ("A2", (2 * C, NBLK)),
        ("A1p", (2 * C, NBLK)),
        ("A2p", (2 * C, NBLK)),
        ("B1", (2 * C, NBLK)),
        ("B2", (2 * C, NBLK)),
        ("R128", (2 * C, NBLK)),
        ("EMF", (2 * C, 256)),
        ("EMB", (2 * C, 256)),
        ("TMF", (L, C * 256)),
        ("TMB", (L, C * 256)),
    ]:
        d[name] = din(name, shape, F32R if name in MM_INPUTS else F32)
    out_d = nc.dram_tensor("out", [4 * C, T], F32, kind="ExternalOutput").ap()

    NCHUNK = 8  # channels per TM chunk
    CHW = NCHUNK * 256

    from contextlib import ExitStack
    with TC(nc) as tc, ExitStack() as es:
        consts = es.enter_context(tc.tile_pool(name="consts", bufs=1))
        tm_pool = es.enter_context(tc.tile_pool(name="tm", bufs=tm_bufs))
        scr = es.enter_context(tc.tile_pool(name="scr", bufs=1))
        dpsum = es.enter_context(tc.tile_pool(name="dpsum", bufs=2, space="PSUM"))
        mpsum = es.enter_context(tc.tile_pool(name="mpsum", bufs=6, space="PSUM"))
        drain = es.enter_context(tc.tile_pool(name="drain", bufs=8))

        # ---- load constants
        ct = {}
        for name in [
            "XF", "XBM", "XBA", "AGG1", "AGG2",
            "A1", "A2", "A1p", "A2p", "B1", "B2", "R128",
        ]:
            shape = list(d[name].shape)
            pad = name in ("XF", "XBM", "XBA")
            if pad:
                # f32r matmuls need an even moving free size; the aggregate
                # matmuls stream the whole X tile, so pad 375 -> 376 cols.
                shape = [shape[0], 376]
            t = consts.tile(shape, F32R if name in MM_INPUTS else F32, tag=name)
            if pad:
                nc.vector.memset(t[:, 375:376].bitcast(F32), 0.0)
                nc.gpsimd.dma_start(t[:, 0:375], d[name][:])
            else:
                nc.gpsimd.dma_start(t[:], d[name][:])
            ct[name] = t

        # TM chunk tiles: 8 fwd + 8 bwd, each (L, 8*256)
        tm_tiles = {}
        for dirn, src in (("f", d["TMF"]), ("b", d["TMB"])):
            for k in range(C // NCHUNK):
                t = tm_pool.tile([L, CHW], F32R, tag="tm")
                nc.gpsimd.dma_start(t[:], src[:, k * CHW : (k + 1) * CHW])
                tm_tiles[(dirn, k)] = t

        # ---- carries per direction
        cw_tiles = {}
        for dirn in ("f", "b") if not skip_carries else ():
            xagg = ct["XF"] if dirn == "f" else ct["XBA"]
            dps1 = dpsum.tile([2 * C, 376], F32, tag="dps")
            nc.tensor.matmul(
                dps1[:], ct["AGG1"][:], xagg[:], start=True, stop=True
            )
            D1 = scr.tile([2 * C, NBLK], F32, tag=f"D1{dirn}")
            nc.vector.tensor_copy(D1[:], dps1[:, 0:NBLK])
            dps2 = dpsum.tile([2 * C, 376], F32, tag="dps")
            nc.tensor.matmul(
                dps2[:], ct["AGG2"][:], xagg[:], start=True, stop=True
            )
            D2 = scr.tile([2 * C, NBLK], F32, tag=f"D2{dirn}")
            nc.vector.tensor_copy(D2[:], dps2[:, 0:NBLK])

            t1 = scr.tile([2 * C, NBLK], F32, tag="t1")
            t2 = scr.tile([2 * C, NBLK], F32, tag="t2")
            s1in = scr.tile([2 * C, NBLK], F32, tag=f"s1in{dirn}")
            s2in = scr.tile([2 * C, NBLK], F32, tag=f"s2in{dirn}")
            nc.vector.tensor_mul(t1[:], ct["A1"][:], D1[:])
            nc.vector.tensor_mul(t2[:], ct["A2"][:], D2[:])
            nc.vector.tensor_add(s1in[:], t1[:], t2[:])
            nc.vector.tensor_mul(t1[:], ct["A1p"][:], D1[:])
            nc.vector.tensor_mul(t2[:], ct["A2p"][:], D2[:])
            nc.vector.tensor_add(s2in[:], t1[:], t2[:])

            s1o = scr.tile([2 * C, NBLK], F32, tag=f"s1o{dirn}")
            s2o = scr.tile([2 * C, NBLK], F32, tag=f"s2o{dirn}")
            nc.vector.tensor_tensor_scan(
                s1o[:], ct["R128"][:], s1in[:], 0.0,
                mybir.AluOpType.mult, mybir.AluOpType.add,
            )
            nc.vector.tensor_tensor_scan(
                s2o[:], ct["R128"][:], s2in[:], 0.0,
                mybir.AluOpType.mult, mybir.AluOpType.add,
            )

            u1 = scr.tile([2 * C, NBLK], F32, tag="t1")
            u2 = scr.tile([2 * C, NBLK], F32, tag="t2")
            cwraw = scr.tile([2 * C, NBLK], F32, tag=f"cwraw{dirn}")
            nc.vector.tensor_mul(u1[:], ct["B1"][:], s1o[:])
            nc.vector.tensor_mul(u2[:], ct["B2"][:], s2o[:])
            nc.vector.tensor_add(cwraw[:], u1[:], u2[:])

            cw = scr.tile([2 * C, 376], F32, tag=f"cw{dirn}")
            if dirn == "f":
                # CW[:, 1+j] = c_j; col 0 = 0 (zero initial carry)
                nc.vector.memset(cw[:, 0:1], 0.0)
                nc.vector.tensor_copy(cw[:, 1:376], cwraw[:])
            else:
                # CW[:, q] = c_rev[373-q] for q=0..373; cols 374,375 = 0
                nc.vector.memset(cw[:, 374:376], 0.0)
                nc.vector.tensor_copy(cw[:, 0:374], cwraw[:, 0:374][:, ::-1])
            cw_tiles[dirn] = cw

        # ---- main matmuls
        # PE operands must start at 32-aligned partitions, so each channel's
        # (carry rows || EM rows) pair is staged into a base-0 tile first.
        pair_pool = es.enter_context(tc.tile_pool(name="pair", bufs=8))
        for dirn in ("f", "b") if not skip_mains else ():
            xm = ct["XF"] if dirn == "f" else ct["XBM"]
            em_d = d["EMF"] if dirn == "f" else d["EMB"]
            cw = cw_tiles.get(dirn)
            row0 = 0 if dirn == "f" else 2 * C
            for c in range(nch):
                pair = pair_pool.tile([2, 376 + 256], F32R, tag="pair")
                if not skip_carries:
                    nc.gpsimd.dma_start(
                        pair[:, 0:376], cw[2 * c : 2 * c + 2, :].bitcast(F32R)
                    )
                else:
                    nc.vector.memset(pair[:, 0:376].bitcast(F32), 0.0)
                nc.gpsimd.dma_start(pair[:, 376:632], em_d[2 * c : 2 * c + 2, :])
                tmt = tm_tiles[(dirn, c // NCHUNK)]
                rhs = tmt[:, (c % NCHUNK) * 256 : (c % NCHUNK) * 256 + 256]
                for g in range(NG):
                    ps = mpsum.tile([JG, 256], F32, tag="mps")
                    nc.tensor.matmul(
                        ps[:],
                        xm[:, JG * g : JG * g + JG],
                        rhs,
                        start=True, stop=skip_rank2,
                    )
                    if not skip_rank2:
                        nc.tensor.matmul(
                            ps[:],
                            pair[:, JG * g : JG * g + JG],
                            pair[:, 376:632],
                            start=False, stop=True, tile_position=(0, 0),
                        )
                    sb = drain.tile([JG, 256], F32, tag="drain")
                    getattr(nc, copy_engine).tensor_copy(sb[:], ps[:])
                    dst_re = out_d[
                        row0 + c : row0 + c + 1, 16000 * g : 16000 * (g + 1)
                    ].rearrange("o (p f) -> (o p) f", p=JG)
                    dst_im = out_d[
                        row0 + C + c : row0 + C + c + 1,
                        16000 * g : 16000 * (g + 1),
                    ].rearrange("o (p f) -> (o p) f", p=JG)
                    nc.sync.dma_start(dst_re, sb[:, 0:128])
                    nc.sync.dma_start(dst_im, sb[:, 128:256])

    return nc


_CACHE = {}


def _get_program():
    if "nc" not in _CACHE:
        _CACHE["nc"] = build_nc()
        _CACHE["static"] = build_static()
    return _CACHE["nc"], _CACHE["static"]


def last_exec_time_ns():
    return _CACHE.get("exec_time_ns")


def kernel(audio, kernels_re=None, kernels_im=None):
    import os

    audio = np.asarray(audio, np.float32)
    assert audio.shape == (N_CORES, T), audio.shape
    nc, static = _get_program()
    in_maps = []
    for b in range(N_CORES):
        m = dict(static)
        m.update(per_core_inputs(audio[b]))
        in_maps.append(m)
    trace = bool(os.environ.get("CSPACE_TRACE"))
    res = run_bass_kernel_spmd(
        nc, in_maps, list(range(N_CORES)), trace=trace
    )
    _CACHE["exec_time_ns"] = res.exec_time_ns
    _CACHE["last_results"] = res
    return np.stack([res.results[b]["out"] for b in range(N_CORES)], 0)



# revision 11
# speedup vs baseline: 1.7551x; 1.7551x over previous
"""Trainium2 Bass kernel for nn_CSpace: bank of 64 complex one-pole resonators
applied to audio forward and backward (FFT convolution in the reference).

The resonator kernels are exact geometric sequences, so the convolution is a
first-order complex IIR computed as block-Toeplitz matmuls plus a rank-2
cross-block carry:

  - time is split into 375 blocks of L=128, grouped as 128/128/119 blocks so
    each matmul uses the full 128-partition output dim,
  - all PE operands are fp16 (1 cycle/row vs 4 for fp32-HIGH); psum stays f32,
  - two channels are packed per matmul (512-wide moving operand): one
    128x(128|120) @ (128x512) main matmul + one rank-4 carry matmul with a
    block-diagonal E tile accumulate into the same PSUM tile,
  - the cross-block carry state is a trig-decomposed REAL first-order scan
    (tensor_tensor_scan) over per-block aggregates from two small matmuls,
  - PSUM tiles drain via Scalar/Vector copies into rotating (128,2048)
    staging tiles; each full tile leaves as ONE ~1MB DMA issued on GpSimd
    (48 total vs 768 small Sync DMAs). SWDGE (gpsimd) descriptor streams
    round-robin across all 16 DMA engines (~265 GB/s), while Sync's HWDGE
    queue only reaches 5 engines (~80 GB/s) - measured, the baseline's real
    bottleneck,
  - backward direction reuses the machinery on host-reversed views with
    column-flipped static matrices so outputs stream out in natural order.

Per core = one batch element: 64ch x 2dir x {Re,Im} x 48000 outputs.
"""
import sys

sys.path.insert(0, "/opt/trn_rl_repo")

import numpy as np

import concourse.bass as bass
import concourse.tile as tile
from concourse import mybir
from concourse.bass_utils import run_bass_kernel_spmd
from concourse.vector_clock import ScopedClock

# ---------------------------------------------------------------- constants
T = 48000
L = 128
NBLK = 375  # T / L
C = 64  # channels
N_CORES = 8
# block-group j ranges: (j0, j_mm, j_out): matmul cols and drained rows
GROUPS = ((0, 128, 128), (128, 128, 128), (256, 120, 119))

RES_SIZE = 64
SR = 24000
MIN_FREQ = 10.0
MAX_FREQ = 12000.0
DECAY_FACTOR = 0.999
HIGH_FREQ_DECAY = 0.6

F32 = mybir.dt.float32
F16 = mybir.dt.float16


class TC(tile.TileContext):
    """TileContext adapted to this walrus build: TPB instruction structs
    accept only ONE sync-wait command, so any scheduled instruction carrying
    more gets its extra waits split onto preceding same-engine nops (and the
    kernel-tail drain gets the same treatment)."""

    def _engine_for(self, engine):
        nc = self.nc
        m = {
            mybir.EngineType.PE: nc.tensor,
            mybir.EngineType.DVE: nc.vector,
            mybir.EngineType.Activation: nc.scalar,
            mybir.EngineType.Pool: nc.gpsimd,
            mybir.EngineType.SP: nc.sync,
        }
        return m[engine]

    def _commit_instruction(self, inst, lazy_reg_writes: bool = True):
        si = getattr(inst, "sync_info", None)
        if (
            si is not None
            and si.on_wait
            and len(si.on_wait) > 1
            and inst.engine in (
                mybir.EngineType.PE,
                mybir.EngineType.DVE,
                mybir.EngineType.Activation,
                mybir.EngineType.Pool,
                mybir.EngineType.SP,
            )
        ):
            waits = list(si.on_wait)
            cb = self.nc._state.pop_inst_callback()
            try:
                eng = self._engine_for(inst.engine)
                for w in waits[1:]:
                    nop = eng.nop()
                    nop.ins.sync_info = mybir.SyncInfo(
                        on_wait=[w], on_update=[]
                    )
            finally:
                self.nc._state.push_inst_callback(cb)
            si.on_wait = waits[:1]
        super()._commit_instruction(inst, lazy_reg_writes)

    def _drain_and_barrier(self, tick_clock, wait_clock):
        nc = self.nc
        probe = nc.sync.nop()
        wait_clock.add_sem_waits(
            probe.ins, ScopedClock({None: tick_clock.global_clock})
        )
        waits = list(probe.ins.sync_info.on_wait or [])
        probe.ins.sync_info.on_wait = waits[:1]
        for i in range(1, len(waits)):
            nop = nc.sync.nop()
            wait_clock.add_sem_waits(
                nop.ins, ScopedClock({None: tick_clock.global_clock})
            )
            nop.ins.sync_info.on_wait = waits[i : i + 1]
        nc.sync.drain()
        nc.all_engine_barrier()
        popped = nc._tile_sem_poison_stack.pop()
        assert popped is self._sem_poison
        nc.clear_and_free_semaphores(list(self.sems.allocated().values()))
        nc.all_engine_barrier()


# ---------------------------------------------------------------- host math
def _channel_params():
    freqs = np.logspace(np.log10(MIN_FREQ), np.log10(MAX_FREQ), RES_SIZE)
    decays = np.linspace(DECAY_FACTOR, HIGH_FREQ_DECAY, RES_SIZE)
    thetas = 2 * np.pi * freqs / SR
    eig = decays * np.exp(1j * thetas)
    gain = (1 - decays) / (1 - decays**SR)  # 1 / sum_t |eig|^t
    return eig, decays, thetas, gain


def build_static():
    """All data-independent arrays. PE operands in fp16, DVE arrays f32."""
    eig, r, thetas, gain = _channel_params()

    # powers[c, t] = eig_c^t for t in 0..L
    tpow = np.arange(L + 1)
    powers = eig[:, None] ** tpow[None, :]  # (C, L+1)

    i_idx = np.arange(L)
    dif = i_idx[:, None] - i_idx[None, :]  # (i, m)
    mask = dif >= 0
    gath = np.clip(dif, 0, None)

    TMF = np.zeros((C, L, 256), np.float64)
    TMB = np.zeros((C, L, 256), np.float64)
    EMF = np.zeros((2 * C, 256), np.float64)
    EMB = np.zeros((2 * C, 256), np.float64)
    for c in range(C):
        pw = powers[c][gath]  # (i, m) complex
        Tre = np.where(mask, gain[c] * pw.real, 0.0)
        Tim = np.where(mask, gain[c] * pw.imag, 0.0)
        TMF[c, :, :128] = Tre.T  # [m, i]
        TMF[c, :, 128:] = Tim.T
        TMB[c, :, :128] = Tre.T[:, ::-1]
        TMB[c, :, 128:] = Tim.T[:, ::-1]
        e = powers[c][1 : L + 1]  # eig^{i+1}, i = 0..127
        EMF[2 * c, :128], EMF[2 * c, 128:] = e.real, e.imag
        EMF[2 * c + 1, :128], EMF[2 * c + 1, 128:] = -e.imag, e.real
        eb = powers[c][L - np.arange(L)]  # eig^{128-i'}
        EMB[2 * c, :128], EMB[2 * c, 128:] = eb.real, eb.imag
        EMB[2 * c + 1, :128], EMB[2 * c + 1, 128:] = -eb.imag, eb.real

    m_idx = np.arange(L)
    AGG1 = np.zeros((L, 128), np.float64)
    AGG2 = np.zeros((L, 128), np.float64)
    for c in range(C):
        base = gain[c] * r[c] ** (127 - m_idx)
        cosm = np.cos(thetas[c] * m_idx)
        sinm = np.sin(thetas[c] * m_idx)
        AGG1[:, c] = base * cosm
        AGG1[:, 64 + c] = base * sinm
        AGG2[:, c] = base * sinm
        AGG2[:, 64 + c] = base * cosm

    j_idx = np.arange(NBLK)
    ang_b = thetas[:, None] * (128.0 * j_idx[None, :])  # (C, NBLK)
    ang_e = thetas[:, None] * (128.0 * j_idx[None, :] + 127.0)
    COSB, SINB = np.cos(ang_b), np.sin(ang_b)
    COSB2, SINB2 = np.cos(ang_e), np.sin(ang_e)
    A1 = np.concatenate([COSB, COSB], 0)
    A2 = np.concatenate([-SINB, SINB], 0)
    A1p = np.concatenate([SINB, -SINB], 0)
    A2p = np.concatenate([COSB, COSB], 0)
    B1 = np.concatenate([COSB2, -COSB2], 0)
    B2 = np.concatenate([SINB2, SINB2], 0)
    R128 = np.repeat((r**128)[None, :], 2, 0).reshape(2 * C, 1) * np.ones((1, NBLK))

    # Interleave chain rows so carries come out as (cr_c, ci_c) at rows
    # (2c, 2c+1): row 2c <- old row c, row 2c+1 <- old row 64+c. The same
    # permutation applies to AGG columns (they define the chain order).
    perm = np.empty(2 * C, np.int64)
    perm[0::2] = np.arange(C)
    perm[1::2] = np.arange(C) + C
    AGG1 = AGG1[:, perm]
    AGG2 = AGG2[:, perm]
    A1, A2, A1p, A2p = A1[perm], A2[perm], A1p[perm], A2p[perm]
    B1, B2, R128 = B1[perm], B2[perm], R128[perm]

    # E tiles for the rank-4 pair carry matmuls: per pair c2 a (4, 512)
    # block-diagonal window [rows 0,1: even ch in cols 0:256; rows 2,3: odd
    # ch in cols 256:512], laid side by side -> (4, 32*512).
    EF2 = np.zeros((4, 32 * 512), np.float64)
    EB2 = np.zeros((4, 32 * 512), np.float64)
    for c2 in range(32):
        w = 512 * c2
        EF2[0:2, w : w + 256] = EMF[4 * c2 : 4 * c2 + 2]
        EF2[2:4, w + 256 : w + 512] = EMF[4 * c2 + 2 : 4 * c2 + 4]
        EB2[0:2, w : w + 256] = EMB[4 * c2 : 4 * c2 + 2]
        EB2[2:4, w + 256 : w + 512] = EMB[4 * c2 + 2 : 4 * c2 + 4]

    f16, f32 = np.float16, np.float32
    return {
        "TMF": np.ascontiguousarray(
            TMF.transpose(1, 0, 2).reshape(L, C * 256)
        ).astype(f16),
        "TMB": np.ascontiguousarray(
            TMB.transpose(1, 0, 2).reshape(L, C * 256)
        ).astype(f16),
        "EF2": EF2.astype(f16),
        "EB2": EB2.astype(f16),
        "AGG1": AGG1.astype(f16),
        "AGG2": AGG2.astype(f16),
        "A1": A1.astype(f32),
        "A2": A2.astype(f32),
        "A1p": A1p.astype(f32),
        "A2p": A2p.astype(f32),
        "B1": B1.astype(f32),
        "B2": B2.astype(f32),
        "R128": R128.astype(f32),
    }


def per_core_inputs(x):
    """x: (48000,) audio for one batch element -> fp16 (128, 376) views
    with a zero pad column (f16 matmuls need an even moving free size and
    the padded block keeps group 2's stationary free size even)."""
    XFn = np.ascontiguousarray(x.reshape(NBLK, L).T).astype(np.float32)
    out = {}
    for name, arr in (
        ("XF", XFn),
        ("XBM", XFn[::-1, :]),
        ("XBA", XFn[::-1, ::-1]),
    ):
        p = np.zeros((L, NBLK + 1), np.float16)
        p[:, :NBLK] = arr.astype(np.float16)
        out[name] = p
    return out


# ---------------------------------------------------------------- program
def build_nc(stg_bufs=4, copy_cycle=("scalar", "vector")):
    nc = bass.Bass(
        trn_type="TRN2", target_bir_lowering=False, debug=False, num_devices=1
    )

    def din(name, shape, dt):
        return nc.dram_tensor(name, list(shape), dt, kind="ExternalInput").ap()

    d = {}
    for name, shape, dt in [
        ("XF", (L, 376), F16),
        ("XBM", (L, 376), F16),
        ("XBA", (L, 376), F16),
        ("AGG1", (L, 128), F16),
        ("AGG2", (L, 128), F16),
        ("A1", (2 * C, NBLK), F32),
        ("A2", (2 * C, NBLK), F32),
        ("A1p", (2 * C, NBLK), F32),
        ("A2p", (2 * C, NBLK), F32),
        ("B1", (2 * C, NBLK), F32),
        ("B2", (2 * C, NBLK), F32),
        ("R128", (2 * C, NBLK), F32),
        ("EF2", (4, 32 * 512), F16),
        ("EB2", (4, 32 * 512), F16),
        ("TMF", (L, C * 256), F16),
        ("TMB", (L, C * 256), F16),
    ]:
        d[name] = din(name, shape, dt)
    out_d = nc.dram_tensor("out", [4 * C, T], F32, kind="ExternalOutput").ap()

    from contextlib import ExitStack
    with TC(nc) as tc, ExitStack() as es:
        consts = es.enter_context(tc.tile_pool(name="consts", bufs=1))
        tm_pool = es.enter_context(tc.tile_pool(name="tm", bufs=2))
        e_pool = es.enter_context(tc.tile_pool(name="ep", bufs=1))
        scr = es.enter_context(tc.tile_pool(name="scr", bufs=1))
        cw16p = es.enter_context(tc.tile_pool(name="cw16", bufs=2))
        pairp = es.enter_context(tc.tile_pool(name="pair", bufs=4))
        dpsum = es.enter_context(tc.tile_pool(name="dpsum", bufs=2, space="PSUM"))
        mpsum = es.enter_context(tc.tile_pool(name="mpsum", bufs=6, space="PSUM"))
        stg = es.enter_context(tc.tile_pool(name="stg", bufs=stg_bufs))

        # ---- load constants
        ct = {}
        for name in [
            "XF", "XBM", "XBA", "AGG1", "AGG2",
            "A1", "A2", "A1p", "A2p", "B1", "B2", "R128",
        ]:
            t = consts.tile(
                list(d[name].shape),
                F16 if name in ("XF", "XBM", "XBA", "AGG1", "AGG2") else F32,
                tag=name,
            )
            nc.gpsimd.dma_start(t[:], d[name][:])
            ct[name] = t

        tm_tiles = {}
        for dirn, tms in (("f", d["TMF"]), ("b", d["TMB"])):
            t = tm_pool.tile([L, C * 256], F16, tag="tm")
            nc.gpsimd.dma_start(t[:], tms[:])
            tm_tiles[dirn] = t

        # ---- carries per direction: (2C, 376) f32, col j = carry into blk j
        cw16_tiles = {}
        for dirn in ("f", "b"):
            xagg = ct["XF"] if dirn == "f" else ct["XBA"]
            dps1 = dpsum.tile([2 * C, 376], F32, tag="dps")
            nc.tensor.matmul(
                dps1[:], ct["AGG1"][:], xagg[:], start=True, stop=True
            )
            D1 = scr.tile([2 * C, NBLK], F32, tag=f"D1{dirn}")
            nc.vector.tensor_copy(D1[:], dps1[:, 0:NBLK])
            dps2 = dpsum.tile([2 * C, 376], F32, tag="dps")
            nc.tensor.matmul(
                dps2[:], ct["AGG2"][:], xagg[:], start=True, stop=True
            )
            D2 = scr.tile([2 * C, NBLK], F32, tag=f"D2{dirn}")
            nc.vector.tensor_copy(D2[:], dps2[:, 0:NBLK])

            t1 = scr.tile([2 * C, NBLK], F32, tag="t1")
            t2 = scr.tile([2 * C, NBLK], F32, tag="t2")
            s1in = scr.tile([2 * C, NBLK], F32, tag=f"s1in{dirn}")
            s2in = scr.tile([2 * C, NBLK], F32, tag=f"s2in{dirn}")
            nc.vector.tensor_mul(t1[:], ct["A1"][:], D1[:])
            nc.vector.tensor_mul(t2[:], ct["A2"][:], D2[:])
            nc.vector.tensor_add(s1in[:], t1[:], t2[:])
            nc.vector.tensor_mul(t1[:], ct["A1p"][:], D1[:])
            nc.vector.tensor_mul(t2[:], ct["A2p"][:], D2[:])
            nc.vector.tensor_add(s2in[:], t1[:], t2[:])

            s1o = scr.tile([2 * C, NBLK], F32, tag=f"s1o{dirn}")
            s2o = scr.tile([2 * C, NBLK], F32, tag=f"s2o{dirn}")
            nc.vector.tensor_tensor_scan(
                s1o[:], ct["R128"][:], s1in[:], 0.0,
                mybir.AluOpType.mult, mybir.AluOpType.add,
            )
            nc.vector.tensor_tensor_scan(
                s2o[:], ct["R128"][:], s2in[:], 0.0,
                mybir.AluOpType.mult, mybir.AluOpType.add,
            )

            u1 = scr.tile([2 * C, NBLK], F32, tag="t1")
            u2 = scr.tile([2 * C, NBLK], F32, tag="t2")
            cwraw = scr.tile([2 * C, NBLK], F32, tag=f"cwraw{dirn}")
            nc.vector.tensor_mul(u1[:], ct["B1"][:], s1o[:])
            nc.vector.tensor_mul(u2[:], ct["B2"][:], s2o[:])
            nc.vector.tensor_add(cwraw[:], u1[:], u2[:])

            cw = scr.tile([2 * C, 376], F32, tag=f"cw{dirn}")
            if dirn == "f":
                # CW[:, 1+j] = c_j; col 0 = 0 (zero initial carry)
                nc.vector.memset(cw[:, 0:1], 0.0)
                nc.vector.tensor_copy(cw[:, 1:376], cwraw[:])
            else:
                # CW[:, q] = c_rev[373-q] for q=0..373; cols 374,375 = 0
                nc.vector.memset(cw[:, 374:376], 0.0)
                nc.vector.tensor_copy(cw[:, 0:374], cwraw[:, 0:374][:, ::-1])
            cw16 = cw16p.tile([2 * C, 384], F16, tag="cw16")
            nc.vector.tensor_copy(cw16[:, 0:376], cw[:])
            nc.vector.memset(cw16[:, 376:384], 0.0)
            cw16_tiles[dirn] = cw16

        # ---- main matmuls + carry accumulate, drain to staging, big DMAs
        n_copy = len(copy_cycle)
        cidx = 0
        for dirn in ("f", "b"):
            xm = ct["XF"] if dirn == "f" else ct["XBM"]
            tmt = tm_tiles[dirn]
            ept = e_pool.tile([4, 32 * 512], F16, tag="ep")
            nc.gpsimd.dma_start(
                ept[:], d["EF2" if dirn == "f" else "EB2"][:]
            )
            cw16 = cw16_tiles[dirn]
            row0 = 0 if dirn == "f" else 2 * C
            for (j0, j_mm, j_out) in GROUPS:
                for c2 in range(32):
                    # stage the pair's carry rows to a partition-0 tile
                    # (PE operands must be 32-aligned; engine copies can't
                    # read partition 4*c2 either, so stage via Sync DMA)
                    pair = pairp.tile([4, 384], F16, tag="pair")
                    nc.sync.dma_start(
                        pair[:], cw16[4 * c2 : 4 * c2 + 4, :]
                    )
                    ps = mpsum.tile([L, 512], F32, tag="mps")
                    nc.tensor.matmul(
                        ps[0:j_mm, :],
                        xm[:, j0 : j0 + j_mm],
                        tmt[:, 512 * c2 : 512 * c2 + 512],
                        start=True, stop=False,
                    )
                    nc.tensor.matmul(
                        ps[0:j_mm, :],
                        pair[:, j0 : j0 + j_mm],
                        ept[:, 512 * c2 : 512 * c2 + 512],
                        start=False, stop=True,
                    )
                    # drain into the staging slot (8 pairs per slot)
                    eighth = c2 % 8
                    if eighth == 0:
                        slot = stg.tile([L, 4096], F32, tag="stg")
                    ename = copy_cycle[cidx % n_copy]
                    cidx += 1
                    dst = slot[0:j_out, 512 * eighth : 512 * eighth + 512]
                    if ename == "scalar":
                        nc.scalar.copy(dst, ps[0:j_out, :])
                    else:
                        getattr(nc, ename).tensor_copy(dst, ps[0:j_out, :])
                    if eighth == 7:
                        c0 = 16 * (c2 // 8)  # first channel of the slot
                        tlen = 128 * j_out
                        # one DMA per plane: 3-dim APs (j, c, i) both sides
                        vall = out_d[
                            row0 : row0 + 2 * C, 16384 * (j0 // 128) :
                            16384 * (j0 // 128) + tlen
                        ].rearrange(
                            "(pl c) (j i) -> j c pl i", pl=2, i=128
                        )[:, c0 : c0 + 16]
                        sall = slot[0:j_out, :].rearrange(
                            "j (c pl i) -> j c pl i", pl=2, i=128
                        )
                        for pl in range(2):
                            nc.gpsimd.dma_start(
                                vall[:, :, pl, :], sall[:, :, pl, :]
                            )

    return nc


_CACHE = {}


def _get_program():
    if "nc" not in _CACHE:
        _CACHE["nc"] = build_nc()
        _CACHE["static"] = build_static()
    return _CACHE["nc"], _CACHE["static"]


def last_exec_time_ns():
    return _CACHE.get("exec_time_ns")


def kernel(audio, kernels_re=None, kernels_im=None):
    import os

    audio = np.asarray(audio, np.float32)
    assert audio.shape == (N_CORES, T), audio.shape
    nc, static = _get_program()
    in_maps = []
    for b in range(N_CORES):
        m = dict(static)
        m.update(per_core_inputs(audio[b]))
        in_maps.append(m)
    trace = bool(os.environ.get("CSPACE_TRACE"))
    res = run_bass_kernel_spmd(
        nc, in_maps, list(range(N_CORES)), trace=trace
    )
    _CACHE["exec_time_ns"] = res.exec_time_ns
    _CACHE["last_results"] = res
    return np.stack([res.results[b]["out"] for b in range(N_CORES)], 0)
